# revision 7
# baseline (speedup 1.0000x reference)
"""Multi-head attention (B=2, T=4096, D=512, H=8) on 8 TRN2 NeuronCores.

Sharding: core c handles batch c//4 and query rows (c%4)*1024..+1024.
Heads stay together on a core; K/V are recomputed per core (no comm).
The host ROLLS the token axis per core so the core's query block sits at
columns 0..1024 of xT (keys are consistently permuted; softmax + PV are
permutation-invariant over keys, so the output is unchanged).  This lets
one shared program serve all 8 cores without a separate xQT input.

v2 - single fused phase, engineered around three measured bottlenecks of
the phase-split baseline (411 us):
  * ACT exp is the steady-state pacer (~1.06 us per key-chunk iteration);
    everything else must hide under it.  ACT does ONLY exp - all copies
    moved to GpSimd(Pool)/DVE.
  * The PE p-state ramp (2.4 GHz only after ~3 us of gapless execution)
    punishes stalls.  Projections for later head-pairs and the V tiles
    are injected INTO the attention instruction stream so the PE queue
    never drains: pair order alternates query blocks so only fo=0 K/Q
    is needed up front.
  * Pair-boundary normalize stalls: O^T PSUM accumulators are double
    buffered (4 banks) and the normalize chain is DVE recip straight
    from PSUM + stream_shuffle partition broadcast + DVE mul - no ACT,
    no PE broadcast matmul, no extra PSUM bank.
PSUM = 8 banks exactly: 3-slot [128,512] ring for per-head S tiles (the
pair-0 V fillers ride this ring), a 1-bank filler tag for interleaved
K/Q/O-proj tiles, and 2 pairs x 2 heads of [65,512] O^T accumulators.
"""

import sys

sys.path.insert(0, "/opt/trn_rl_repo")

import numpy as np
import ml_dtypes

B, T, D, H = 2, 4096, 512, 8
DH = D // H          # 64
N_CORES = 8
QPC = 1024           # query rows per core
DC = D // 128        # 4 partition chunks of the model dim
KC = T // 128        # 32 key chunks

_BUILT = {}


def _build(with_bias: bool):
    from concourse import bacc
    import concourse.mybir as mybir
    import concourse.tile as tile

    dt = mybir.dt
    AF = mybir.ActivationFunctionType

    nc = bacc.Bacc("TRN2", target_bir_lowering=False, debug=False,
                   num_devices=N_CORES)

    xT = nc.dram_tensor("xT", [128, DC, T], dt.bfloat16, kind="ExternalInput").ap()
    wq = nc.dram_tensor("wq", [128, DC, D], dt.bfloat16, kind="ExternalInput").ap()
    wk = nc.dram_tensor("wk", [128, DC, D], dt.bfloat16, kind="ExternalInput").ap()
    wv = nc.dram_tensor("wv", [128, DC, D], dt.bfloat16, kind="ExternalInput").ap()
    wo = nc.dram_tensor("wo", [128, DC, D], dt.bfloat16, kind="ExternalInput").ap()
    msk = nc.dram_tensor("msk", [128, KC, QPC], dt.bfloat16, kind="ExternalInput").ap()
    if with_bias:
        bqkd = nc.dram_tensor("bqk", [128, DC, 2], dt.float32, kind="ExternalInput").ap()
        bvo = nc.dram_tensor("bvo", [1, 2, D], dt.bfloat16, kind="ExternalInput").ap()
        ones1 = nc.dram_tensor("ones1", [1, 128], dt.bfloat16, kind="ExternalInput").ap()
    out = nc.dram_tensor("out", [QPC, D], dt.float32, kind="ExternalOutput").ap()

    with tile.TileContext(nc) as tc:
        with (
            tc.tile_pool(name="persist", bufs=1) as pp,
            tc.tile_pool(name="psR", bufs=3, space="PSUM") as psR,
            tc.tile_pool(name="psO", bufs=2, space="PSUM") as psO,
            tc.tile_pool(name="pB", bufs=4) as pB,
            tc.tile_pool(name="pN", bufs=1) as pN,
            tc.tile_pool(name="pC", bufs=2) as pC,
        ):
            wq_sb = pp.tile([128, DC, D], dt.bfloat16, tag="wq")
            wk_sb = pp.tile([128, DC, D], dt.bfloat16, tag="wk")
            wv_sb = pp.tile([128, DC, D], dt.bfloat16, tag="wv")
            wo_sb = pp.tile([128, DC, D], dt.bfloat16, tag="wo")
            msk_sb = pp.tile([128, KC, QPC], dt.bfloat16, tag="msk")
            xt_sb = pp.tile([128, DC, T], dt.bfloat16, tag="xt")
            kt_f = [pp.tile([128, T], dt.bfloat16, tag=f"kt{fo}", name=f"kt{fo}")
                    for fo in range(DC)]
            qt_f = [pp.tile([128, QPC], dt.bfloat16, tag=f"qt{fo}", name=f"qt{fo}")
                    for fo in range(DC)]
            ot_f = [pp.tile([128, QPC], dt.bfloat16, tag=f"ot{fo}", name=f"ot{fo}")
                    for fo in range(DC)]
            v_c = [pp.tile([128, H, DH + 1], dt.bfloat16, tag=f"v{kc}", name=f"v{kc}")
                   for kc in range(KC)]
            recA = pp.tile([64, 512], dt.float32, tag="recA")
            den0 = pp.tile([1, 512], dt.float32, tag="den0")

            if with_bias:
                bqk_sb = pp.tile([128, DC, 2], dt.float32, tag="bqk")
                bvo_sb = pp.tile([1, 2, D], dt.bfloat16, tag="bvo")
                ones1_sb = pp.tile([1, 128], dt.bfloat16, tag="ones1")

            # ---- input DMAs (issue order = arrival order on the queue) ----
            nc.sync.dma_start(wk_sb[:], wk[:])
            for c4 in range(4):
                ts = slice(c4 * 1024, (c4 + 1) * 1024)
                nc.sync.dma_start(xt_sb[:, :, ts], xT[:, :, ts])
            # first mask pieces early (first mask-mul fires ~15 us in)
            for lo, hi in ((0, 2), (2, 4)):
                nc.sync.dma_start(msk_sb[:, lo:hi, :], msk[:, lo:hi, :])
            nc.sync.dma_start(wq_sb[:], wq[:])
            nc.sync.dma_start(wv_sb[:], wv[:])
            for lo, hi in ((4, 8), (8, 12), (12, 16)):
                nc.sync.dma_start(msk_sb[:, lo:hi, :], msk[:, lo:hi, :])
            nc.sync.dma_start(wo_sb[:], wo[:])
            if with_bias:
                nc.sync.dma_start(bqk_sb[:], bqkd[:])
                nc.sync.dma_start(bvo_sb[:], bvo[:])
                nc.sync.dma_start(ones1_sb[:], ones1[:])
            for lo, hi in ((16, 20), (20, 24), (24, 28), (28, 32)):
                nc.sync.dma_start(msk_sb[:, lo:hi, :], msk[:, lo:hi, :])

            # ones columns of V_aug (denominator accumulators), recA seed
            nc.vector.memset(recA[:], 1.0)
            for kc in range(KC):
                nc.gpsimd.memset(v_c[kc][:, :, DH:DH + 1], 1.0)

            # ---------------- work units -------------------------------
            def kq_unit(w_sb, fo, tok0, out_ap, bi, tag):
                """One [128,512] K^T/Q^T projection chunk; copy off-ACT."""
                ps = psR.tile([128, 512], dt.float32, tag=tag,
                              bufs=(1 if tag == "fl" else None))
                for dc in range(DC):
                    nc.tensor.matmul(
                        ps[:],
                        w_sb[:, dc, fo * 128:(fo + 1) * 128],
                        xt_sb[:, dc, tok0:tok0 + 512],
                        start=(dc == 0), stop=(dc == DC - 1),
                    )
                if with_bias:
                    nc.vector.tensor_scalar(
                        out_ap, ps[:], bqk_sb[:, fo, bi:bi + 1], 0.0,
                        mybir.AluOpType.add, mybir.AluOpType.bypass,
                    )
                else:
                    nc.vector.tensor_copy(out_ap, ps[:])

            def v_unit(tt, on_act=False):
                """V chunk tt: [128 tokens, 512 features] -> v_c[tt]."""
                ps = psR.tile([128, 512], dt.float32, tag="sp")
                for dc in range(DC):
                    nc.tensor.matmul(
                        ps[:],
                        xt_sb[:, dc, tt * 128:(tt + 1) * 128],
                        wv_sb[:, dc, :],
                        start=(dc == 0),
                        stop=(not with_bias and dc == DC - 1),
                    )
                if with_bias:
                    nc.tensor.matmul(ps[:], ones1_sb[:], bvo_sb[:, 0, :],
                                     start=False, stop=True)
                src = ps[:].rearrange("p (h f) -> p h f", h=H)
                if on_act:
                    nc.scalar.copy(v_c[tt][:, :, 0:DH], src)
                else:
                    nc.vector.tensor_copy(v_c[tt][:, :, 0:DH], src)

            def oproj_unit(tt):
                """Output projection for token chunk tt (128 rows)."""
                ps = psR.tile([128, 512], dt.float32, tag="fl", bufs=1)
                for dc in range(DC):
                    nc.tensor.matmul(
                        ps[:],
                        ot_f[dc][:, tt * 128:(tt + 1) * 128],
                        wo_sb[:, dc, :],
                        start=(dc == 0),
                        stop=(with_bias is False and dc == DC - 1))
                if with_bias:
                    nc.tensor.matmul(ps[:], ones1_sb[:],
                                     bvo_sb[:, 1, :], start=False, stop=True)
                os = pC.tile([128, 512], dt.float32, tag="os")
                nc.vector.tensor_copy(os[:], ps[:])
                nc.sync.dma_start(out[tt * 128:(tt + 1) * 128, :], os[:])

            # ---------------- emission schedule ------------------------
            # prologue: K(fo0) all token chunks + Q(fo0, first query half)
            for nb in range(8):
                kq_unit(wk_sb, 0, nb * 512, kt_f[0][:, nb * 512:(nb + 1) * 512],
                        1, "sp")
            kq_unit(wq_sb, 0, 0, qt_f[0][:, 0:512], 0, "sp")

            # pair order: (jq0,pr0),(jq1,pr0),(jq0,pr1),(jq1,pr1), ...
            pairs = [(jq, pr) for pr in range(H // 2) for jq in range(2)]

            def kq_group(fo):
                units = []
                for nb in range(8):
                    units.append(lambda fo=fo, nb=nb: kq_unit(
                        wk_sb, fo, nb * 512,
                        kt_f[fo][:, nb * 512:(nb + 1) * 512], 1, "fl"))
                for nb in range(2):
                    units.append(lambda fo=fo, nb=nb: kq_unit(
                        wq_sb, fo, nb * 512,
                        qt_f[fo][:, nb * 512:(nb + 1) * 512], 0, "fl"))
                return units

            fillers = {
                0: ([lambda: kq_unit(wq_sb, 0, 512, qt_f[0][:, 512:1024], 0, "fl")]
                    + [(lambda tt=tt: v_unit(tt, on_act=(tt % 2 == 1)))
                       for tt in range(KC)]),
                1: kq_group(1),
                2: kq_group(2)[:5],
                3: kq_group(2)[5:],
                4: kq_group(3)[:5],
                5: kq_group(3)[5:],
                6: [],
                7: [lambda tt=tt: oproj_unit(tt) for tt in range(4)],
            }

            for pi, (jq, pr) in enumerate(pairs):
                qs = slice(jq * 512, (jq + 1) * 512)
                pend = list(fillers[pi])
                inject_at = {}
                if pend:
                    step = KC / len(pend)
                    for u, unit in enumerate(pend):
                        inject_at.setdefault(int(u * step), []).append(unit)

                ot_e = psO.tile([DH + 1, 512], dt.float32, tag="ote")
                ot_o = psO.tile([DH + 1, 512], dt.float32, tag="oto")
                for kc in range(KC):
                    ks = slice(kc * 128, (kc + 1) * 128)
                    sp_e = psR.tile([128, 512], dt.float32, tag="sp")
                    sp_o = psR.tile([128, 512], dt.float32, tag="sp")
                    nc.tensor.matmul(sp_e[:], kt_f[pr][0:64, ks],
                                     qt_f[pr][0:64, qs], start=True, stop=True)
                    nc.tensor.matmul(sp_o[:], kt_f[pr][64:128, ks],
                                     qt_f[pr][64:128, qs], start=True, stop=True)
                    for unit in inject_at.get(kc, ()):
                        unit()
                    p_e = pB.tile([128, 512], dt.bfloat16, tag="p")
                    p_o = pB.tile([128, 512], dt.bfloat16, tag="p")
                    mk = msk_sb[:, kc, qs]
                    nc.scalar.activation(p_e[:], sp_e[:], AF.Exp, scale=0.125)
                    nc.vector.tensor_mul(p_e[:], p_e[:], mk)
                    nc.tensor.matmul(ot_e[:], v_c[kc][:, 2 * pr, :], p_e[:],
                                     start=(kc == 0), stop=(kc == KC - 1))
                    nc.scalar.activation(p_o[:], sp_o[:], AF.Exp, scale=0.125)
                    nc.vector.tensor_mul(p_o[:], p_o[:], mk)
                    nc.tensor.matmul(ot_o[:], v_c[kc][:, 2 * pr + 1, :], p_o[:],
                                     start=(kc == 0), stop=(kc == KC - 1))

                # normalize both heads by their denominator row (row 64).
                # custom-DVE recip only works at partition base 0, so stage
                # the denominator down to p0 first; plain DVE copies handle
                # cross-partition offsets exactly.
                for i, ot_ps in ((0, ot_e), (1, ot_o)):
                    nc.vector.tensor_copy(den0[:], ot_ps[DH:DH + 1, :])
                    nc.vector.reciprocal_approx_fast(recA[0:1, :], den0[:])
                    nc.vector.tensor_copy(recA[32:33, :], recA[0:1, :])
                    bcs = pN.tile([64, 512], dt.float32, tag="bcs")
                    nc.vector.stream_shuffle(bcs[:], recA[:], [0] * 32)
                    nc.vector.tensor_mul(
                        ot_f[pr][i * 64:(i + 1) * 64, qs],
                        ot_ps[0:DH, :], bcs[:])

            # tail: output projection of the last query block
            for tt in range(4, 8):
                oproj_unit(tt)

    nc.compile()
    return nc


def _get_nc(with_bias: bool):
    if with_bias not in _BUILT:
        _BUILT[with_bias] = _build(with_bias)
    return _BUILT[with_bias]


def _prep_inputs(x, Wq, bq, Wk, bk, Wv, bv, Wo, bo, mask, with_bias):
    bf16 = ml_dtypes.bfloat16

    shared = {}
    for name, W in (("wq", Wq), ("wk", Wk), ("wv", Wv), ("wo", Wo)):
        shared[name] = np.ascontiguousarray(
            np.asarray(W, np.float32).astype(bf16)
            .reshape(DC, 128, D).transpose(1, 0, 2))
    if with_bias:
        shared["bqk"] = np.ascontiguousarray(np.stack(
            [np.asarray(bq, np.float32).reshape(DC, 128).T,
             np.asarray(bk, np.float32).reshape(DC, 128).T], axis=-1))
        shared["bvo"] = np.ascontiguousarray(np.stack(
            [np.asarray(bv, np.float32), np.asarray(bo, np.float32)]
        ).astype(bf16).reshape(1, 2, D))
        shared["ones1"] = np.ones((1, 128), np.float32).astype(bf16)

    maskT = np.asarray(mask).reshape(T, T).T          # (k, q)
    m01T = maskT.astype(np.float32)

    in_maps = []
    for c in range(N_CORES):
        b, q0 = c // 4, (c % 4) * QPC
        # Roll the token axis so this core's query block sits at columns
        # 0..QPC; keys are consistently permuted (softmax + PV are
        # permutation-invariant over keys); the mask key axis rolls too.
        xTb = np.asarray(x[b], np.float32).T.astype(bf16)     # (D, T)
        xTr = np.roll(xTb, -q0, axis=1)
        m = dict(shared)
        m["xT"] = np.ascontiguousarray(
            xTr.reshape(DC, 128, T).transpose(1, 0, 2))
        mr = np.roll(m01T[:, q0:q0 + QPC], -q0, axis=0)       # (k rolled, q)
        m["msk"] = np.ascontiguousarray(
            mr.reshape(KC, 128, QPC).transpose(1, 0, 2)).astype(bf16)
        in_maps.append(m)
    return in_maps


def _run(inputs, trace=False):
    from concourse.bass_utils import run_bass_kernel_spmd

    with_bias = any(
        float(np.abs(np.asarray(inputs[k], np.float32)).max()) != 0.0
        for k in ("bq", "bk", "bv", "bo"))
    nc = _get_nc(with_bias)
    in_maps = _prep_inputs(
        inputs["x"], inputs["Wq"], inputs["bq"], inputs["Wk"], inputs["bk"],
        inputs["Wv"], inputs["bv"], inputs["Wo"], inputs["bo"],
        inputs["mask"], with_bias)
    res = run_bass_kernel_spmd(nc, in_maps, list(range(N_CORES)), trace=trace)
    O = np.empty((B, T, D), np.float32)
    for c in range(N_CORES):
        b, q0 = c // 4, (c % 4) * QPC
        O[b, q0:q0 + QPC, :] = res.results[c]["out"]
    return O, res


def kernel(**inputs) -> np.ndarray:
    out, _ = _run(inputs, trace=False)
    return out


# revision 10
# speedup vs baseline: 1.1339x; 1.1339x over previous
"""Multi-head attention (B=2, T=4096, D=512, H=8) on 8 TRN2 NeuronCores.

Sharding: core c handles batch c//4 and query rows (c%4)*1024..+1024.
Heads stay together on a core; K/V are recomputed per core (no comm).
The host ROLLS the token axis per core so the core's query block sits at
columns 0..1024 of xT (keys are consistently permuted; softmax + PV are
permutation-invariant over keys, so the output is unchanged).  This lets
one shared program serve all 8 cores without a separate xQT input.

v3 - single fused phase around the measured bottlenecks of the
phase-split baseline (411 us):
  * ACT exp is the steady-state pacer (~0.96 us per [128,1024] score
    tile); ACT does ONLY exp - all copies live on DVE (GpSimd cannot
    touch PSUM; custom-DVE ops only work at partition base 0).
  * The PE p-state ramp (2.4 GHz only after ~3 us of gapless execution)
    punishes stalls.  K/Q projections for later head-pairs, the V
    tiles, and the first output projection are injected INTO the
    attention stream; pair order alternates query blocks so only fo=0
    K/Q is needed up front.
  * Pair-boundary stalls: O^T accumulators drain via two immediate DVE
    copies per head (numerator in unnormalized bf16 + denominator row),
    so the banks free in ~1.5 us; the reciprocal + partition broadcast
    (stream_shuffle) + in-place normalize run SBUF-side off the
    critical path.  ote is double buffered; oto single (its drain
    completes before the next pair's first PV_o needs the bank).
PSUM = 8 banks: 2x[128,1024] "sp" ring (S tiles; pair-0 V fillers ride
it), 1x[128,512] "fl" (K/Q + O-proj fillers, self-serializing), 2x ote,
1x oto.
"""

import sys

sys.path.insert(0, "/opt/trn_rl_repo")

import numpy as np
import ml_dtypes

B, T, D, H = 2, 4096, 512, 8
DH = D // H          # 64
N_CORES = 8
QPC = 1024           # query rows per core
DC = D // 128        # 4 partition chunks of the model dim
KC = T // 128        # 32 key chunks

_BUILT = {}


def _build(with_bias: bool):
    from concourse import bacc
    import concourse.mybir as mybir
    import concourse.tile as tile

    dt = mybir.dt
    AF = mybir.ActivationFunctionType

    nc = bacc.Bacc("TRN2", target_bir_lowering=False, debug=False,
                   num_devices=N_CORES)

    xT = nc.dram_tensor("xT", [128, DC, T], dt.bfloat16, kind="ExternalInput").ap()
    wq = nc.dram_tensor("wq", [128, DC, D], dt.bfloat16, kind="ExternalInput").ap()
    wk = nc.dram_tensor("wk", [128, DC, D], dt.bfloat16, kind="ExternalInput").ap()
    wv = nc.dram_tensor("wv", [128, DC, D], dt.bfloat16, kind="ExternalInput").ap()
    wo = nc.dram_tensor("wo", [128, DC, D], dt.bfloat16, kind="ExternalInput").ap()
    msk = nc.dram_tensor("msk", [128, KC, QPC], dt.bfloat16, kind="ExternalInput").ap()
    if with_bias:
        bqkd = nc.dram_tensor("bqk", [128, DC, 2], dt.float32, kind="ExternalInput").ap()
        bvo = nc.dram_tensor("bvo", [1, 2, D], dt.bfloat16, kind="ExternalInput").ap()
        ones1 = nc.dram_tensor("ones1", [1, 128], dt.bfloat16, kind="ExternalInput").ap()
    out = nc.dram_tensor("out", [QPC, D], dt.float32, kind="ExternalOutput").ap()

    with tile.TileContext(nc) as tc:
        with (
            tc.tile_pool(name="persist", bufs=1) as pp,
            tc.tile_pool(name="psR", bufs=2, space="PSUM") as psR,
            tc.tile_pool(name="psO", bufs=1, space="PSUM") as psO,
            tc.tile_pool(name="pB", bufs=3) as pB,
            tc.tile_pool(name="pN", bufs=1) as pN,
            tc.tile_pool(name="pC", bufs=1) as pC,
        ):
            wq_sb = pp.tile([128, DC, D], dt.bfloat16, tag="wq")
            wk_sb = pp.tile([128, DC, D], dt.bfloat16, tag="wk")
            wv_sb = pp.tile([128, DC, D], dt.bfloat16, tag="wv")
            wo_sb = pp.tile([128, DC, D], dt.bfloat16, tag="wo")
            msk_sb = pp.tile([128, KC, QPC], dt.bfloat16, tag="msk")
            xt_sb = pp.tile([128, DC, T], dt.bfloat16, tag="xt")
            kt_f = [pp.tile([128, T], dt.bfloat16, tag=f"kt{fo}", name=f"kt{fo}")
                    for fo in range(DC)]
            qt_f = [pp.tile([128, QPC], dt.bfloat16, tag=f"qt{fo}", name=f"qt{fo}")
                    for fo in range(DC)]
            ot_f = [pp.tile([128, QPC], dt.bfloat16, tag=f"ot{fo}", name=f"ot{fo}")
                    for fo in range(DC)]
            v_c = [pp.tile([128, H, DH + 1], dt.bfloat16, tag=f"v{kc}", name=f"v{kc}")
                   for kc in range(KC)]
            recA = pp.tile([128, 512], dt.float32, tag="recA")
            den0 = pp.tile([1, 512], dt.float32, tag="den0")

            if with_bias:
                bqk_sb = pp.tile([128, DC, 2], dt.float32, tag="bqk")
                bvo_sb = pp.tile([1, 2, D], dt.bfloat16, tag="bvo")
                ones1_sb = pp.tile([1, 128], dt.bfloat16, tag="ones1")

            # ---- input DMAs (issue order = arrival order on the queue) ----
            nc.sync.dma_start(wk_sb[:], wk[:])
            for c4 in range(4):
                ts = slice(c4 * 1024, (c4 + 1) * 1024)
                nc.sync.dma_start(xt_sb[:, :, ts], xT[:, :, ts])
            nc.sync.dma_start(wq_sb[:], wq[:])
            # first mask pieces early (first mask-mul fires ~18 us in)
            for lo, hi in ((0, 2), (2, 4)):
                nc.sync.dma_start(msk_sb[:, lo:hi, :], msk[:, lo:hi, :])
            nc.sync.dma_start(wv_sb[:], wv[:])
            for lo, hi in ((4, 8), (8, 12), (12, 16)):
                nc.sync.dma_start(msk_sb[:, lo:hi, :], msk[:, lo:hi, :])
            nc.sync.dma_start(wo_sb[:], wo[:])
            if with_bias:
                nc.sync.dma_start(bqk_sb[:], bqkd[:])
                nc.sync.dma_start(bvo_sb[:], bvo[:])
                nc.sync.dma_start(ones1_sb[:], ones1[:])
            for lo, hi in ((16, 20), (20, 24), (24, 28), (28, 32)):
                nc.sync.dma_start(msk_sb[:, lo:hi, :], msk[:, lo:hi, :])

            # ones columns of V_aug (denominator accumulators), recA seed
            nc.vector.memset(recA[:], 1.0)
            for kc in range(KC):
                nc.gpsimd.memset(v_c[kc][:, :, DH:DH + 1], 1.0)

            # ---------------- work units -------------------------------
            def kq_unit(w_sb, fo, tok0, out_ap, bi, tag):
                """One [128,512] K^T/Q^T projection chunk; copy on DVE."""
                ps = psR.tile([128, 512], dt.float32, tag=tag,
                              bufs=(1 if tag == "fl" else None))
                for dc in range(DC):
                    nc.tensor.matmul(
                        ps[:],
                        w_sb[:, dc, fo * 128:(fo + 1) * 128],
                        xt_sb[:, dc, tok0:tok0 + 512],
                        start=(dc == 0), stop=(dc == DC - 1),
                    )
                if with_bias:
                    nc.vector.tensor_scalar(
                        out_ap, ps[:], bqk_sb[:, fo, bi:bi + 1], 0.0,
                        mybir.AluOpType.add, mybir.AluOpType.bypass,
                    )
                else:
                    nc.vector.tensor_copy(out_ap, ps[:])

            def v_unit(tt, on_act=False):
                """V chunk tt: [128 tokens, 512 features] -> v_c[tt]."""
                ps = psR.tile([128, 512], dt.float32, tag="sp")
                for dc in range(DC):
                    nc.tensor.matmul(
                        ps[:],
                        xt_sb[:, dc, tt * 128:(tt + 1) * 128],
                        wv_sb[:, dc, :],
                        start=(dc == 0),
                        stop=(not with_bias and dc == DC - 1),
                    )
                if with_bias:
                    nc.tensor.matmul(ps[:], ones1_sb[:], bvo_sb[:, 0, :],
                                     start=False, stop=True)
                src = ps[:].rearrange("p (h f) -> p h f", h=H)
                if on_act:
                    nc.scalar.copy(v_c[tt][:, :, 0:DH], src)
                else:
                    nc.vector.tensor_copy(v_c[tt][:, :, 0:DH], src)

            def oproj_unit(tt):
                """Output projection for token chunk tt (128 rows)."""
                ps = psR.tile([128, 512], dt.float32, tag="fl", bufs=1)
                for dc in range(DC):
                    nc.tensor.matmul(
                        ps[:],
                        ot_f[dc][:, tt * 128:(tt + 1) * 128],
                        wo_sb[:, dc, :],
                        start=(dc == 0),
                        stop=(with_bias is False and dc == DC - 1))
                if with_bias:
                    nc.tensor.matmul(ps[:], ones1_sb[:],
                                     bvo_sb[:, 1, :], start=False, stop=True)
                os = pC.tile([128, 512], dt.float32, tag="os")
                nc.vector.tensor_copy(os[:], ps[:])
                nc.sync.dma_start(out[tt * 128:(tt + 1) * 128, :], os[:])

            # ---------------- emission schedule ------------------------
            # prologue: K+Q for fo0 (pairs 0,1) and fo1 (pairs 2,3).
            # Q chunks first (need only the first xt DMA piece).
            for fo in (0, 1):
                for nb in range(2):
                    kq_unit(wq_sb, fo, nb * 512,
                            qt_f[fo][:, nb * 512:(nb + 1) * 512], 0, "sp")
                for nb in range(8):
                    kq_unit(wk_sb, fo, nb * 512,
                            kt_f[fo][:, nb * 512:(nb + 1) * 512], 1, "sp")

            # pair order: (jq0,pr0),(jq1,pr0),(jq0,pr1),(jq1,pr1), ...
            pairs = [(jq, pr) for pr in range(H // 2) for jq in range(2)]

            def kq_group(fo):
                units = []
                for nb in range(8):
                    units.append(lambda fo=fo, nb=nb: kq_unit(
                        wk_sb, fo, nb * 512,
                        kt_f[fo][:, nb * 512:(nb + 1) * 512], 1, "fl"))
                for nb in range(2):
                    units.append(lambda fo=fo, nb=nb: kq_unit(
                        wq_sb, fo, nb * 512,
                        qt_f[fo][:, nb * 512:(nb + 1) * 512], 0, "fl"))
                return units

            fillers = {
                0: [(lambda tt=tt: v_unit(tt, on_act=(tt % 2 == 1)))
                    for tt in range(KC)],
                2: kq_group(2),
                4: kq_group(3),
                7: [lambda tt=tt: oproj_unit(tt) for tt in range(4)],
            }

            for pi, (jq, pr) in enumerate(pairs):
                qs = slice(jq * 512, (jq + 1) * 512)
                pend = list(fillers.get(pi, ()))
                inject_at = {}
                if pend:
                    step = KC / len(pend)
                    for u, unit in enumerate(pend):
                        inject_at.setdefault(int(u * step), []).append(unit)

                ot_e = psO.tile([DH + 1, 512], dt.float32, tag="ote", bufs=2)
                ot_o = psO.tile([DH + 1, 512], dt.float32, tag="oto", bufs=1)
                for kc in range(KC):
                    ks = slice(kc * 128, (kc + 1) * 128)
                    sp = psR.tile([128, 1024], dt.float32, tag="sp")
                    nc.tensor.matmul(sp[:, 0:512], kt_f[pr][0:64, ks],
                                     qt_f[pr][0:64, qs], start=True, stop=True)
                    nc.tensor.matmul(sp[:, 512:1024], kt_f[pr][64:128, ks],
                                     qt_f[pr][64:128, qs], start=True, stop=True)
                    for unit in inject_at.get(kc, ()):
                        unit()
                    p_sb = pB.tile([128, 1024], dt.bfloat16, tag="p")
                    nc.scalar.activation(p_sb[:], sp[:], AF.Exp, scale=0.125)
                    pv = p_sb[:].rearrange("p (a b) -> p a b", a=2)
                    nc.vector.tensor_mul(
                        pv, pv,
                        msk_sb[:, kc, qs][:, None, :].to_broadcast((128, 2, 512)))
                    nc.tensor.matmul(ot_e[:], v_c[kc][:, 2 * pr, :],
                                     p_sb[:, 0:512],
                                     start=(kc == 0), stop=(kc == KC - 1))
                    nc.tensor.matmul(ot_o[:], v_c[kc][:, 2 * pr + 1, :],
                                     p_sb[:, 512:1024],
                                     start=(kc == 0), stop=(kc == KC - 1))

                # Normalize.  Drain PSUM first: unnormalized numerator (bf16)
                # and denominator row for both heads; the reciprocal +
                # stream_shuffle broadcast + in-place muls then run SBUF-side.
                # oto drains in ops 1-2 so its single bank never stalls the
                # next pair; custom-DVE recip requires partition base 0.
                oslc_e = ot_f[pr][0:64, qs]
                oslc_o = ot_f[pr][64:128, qs]
                bcs = pN.tile([128, 512], dt.float32, tag="bcs")
                nc.vector.tensor_copy(den0[:], ot_o[DH:DH + 1, :])
                nc.vector.tensor_copy(oslc_o, ot_o[0:DH, :])
                nc.vector.reciprocal_approx_fast(recA[0:1, :], den0[:])
                nc.vector.tensor_copy(den0[:], ot_e[DH:DH + 1, :])
                nc.vector.tensor_copy(oslc_e, ot_e[0:DH, :])
                nc.vector.tensor_copy(recA[64:65, :], recA[0:1, :])
                nc.vector.tensor_copy(recA[96:97, :], recA[0:1, :])
                nc.vector.stream_shuffle(bcs[64:128, :], recA[64:128, :],
                                         [0] * 32)
                nc.vector.tensor_mul(oslc_o, oslc_o, bcs[64:128, :])
                nc.vector.reciprocal_approx_fast(recA[0:1, :], den0[:])
                nc.vector.tensor_copy(recA[32:33, :], recA[0:1, :])
                nc.vector.stream_shuffle(bcs[0:64, :], recA[0:64, :],
                                         [0] * 32)
                nc.vector.tensor_mul(oslc_e, oslc_e, bcs[0:64, :])

            # tail: output projection of the last query block
            for tt in range(4, 8):
                oproj_unit(tt)

    nc.compile()
    return nc


def _get_nc(with_bias: bool):
    if with_bias not in _BUILT:
        _BUILT[with_bias] = _build(with_bias)
    return _BUILT[with_bias]


def _prep_inputs(x, Wq, bq, Wk, bk, Wv, bv, Wo, bo, mask, with_bias):
    bf16 = ml_dtypes.bfloat16

    shared = {}
    for name, W in (("wq", Wq), ("wk", Wk), ("wv", Wv), ("wo", Wo)):
        shared[name] = np.ascontiguousarray(
            np.asarray(W, np.float32).astype(bf16)
            .reshape(DC, 128, D).transpose(1, 0, 2))
    if with_bias:
        shared["bqk"] = np.ascontiguousarray(np.stack(
            [np.asarray(bq, np.float32).reshape(DC, 128).T,
             np.asarray(bk, np.float32).reshape(DC, 128).T], axis=-1))
        shared["bvo"] = np.ascontiguousarray(np.stack(
            [np.asarray(bv, np.float32), np.asarray(bo, np.float32)]
        ).astype(bf16).reshape(1, 2, D))
        shared["ones1"] = np.ones((1, 128), np.float32).astype(bf16)

    maskT = np.asarray(mask).reshape(T, T).T          # (k, q)
    m01T = maskT.astype(np.float32)

    in_maps = []
    for c in range(N_CORES):
        b, q0 = c // 4, (c % 4) * QPC
        # Roll the token axis so this core's query block sits at columns
        # 0..QPC; keys are consistently permuted (softmax + PV are
        # permutation-invariant over keys); the mask key axis rolls too.
        xTb = np.asarray(x[b], np.float32).T.astype(bf16)     # (D, T)
        xTr = np.roll(xTb, -q0, axis=1)
        m = dict(shared)
        m["xT"] = np.ascontiguousarray(
            xTr.reshape(DC, 128, T).transpose(1, 0, 2))
        mr = np.roll(m01T[:, q0:q0 + QPC], -q0, axis=0)       # (k rolled, q)
        m["msk"] = np.ascontiguousarray(
            mr.reshape(KC, 128, QPC).transpose(1, 0, 2)).astype(bf16)
        in_maps.append(m)
    return in_maps


def _run(inputs, trace=False):
    from concourse.bass_utils import run_bass_kernel_spmd

    with_bias = any(
        float(np.abs(np.asarray(inputs[k], np.float32)).max()) != 0.0
        for k in ("bq", "bk", "bv", "bo"))
    nc = _get_nc(with_bias)
    in_maps = _prep_inputs(
        inputs["x"], inputs["Wq"], inputs["bq"], inputs["Wk"], inputs["bk"],
        inputs["Wv"], inputs["bv"], inputs["Wo"], inputs["bo"],
        inputs["mask"], with_bias)
    res = run_bass_kernel_spmd(nc, in_maps, list(range(N_CORES)), trace=trace)
    O = np.empty((B, T, D), np.float32)
    for c in range(N_CORES):
        b, q0 = c // 4, (c % 4) * QPC
        O[b, q0:q0 + QPC, :] = res.results[c]["out"]
    return O, res


def kernel(**inputs) -> np.ndarray:
    out, _ = _run(inputs, trace=False)
    return out


# revision 14
# speedup vs baseline: 1.4166x; 1.2493x over previous
"""Multi-head attention (B=2, T=4096, D=512, H=8) on 8 TRN2 NeuronCores.

Sharding: core c handles batch c//4 and query rows (c%4)*1024..+1024.
Heads stay together on a core; K/V are recomputed per core (no comm).
The host ROLLS the token axis per core so the core's query block sits at
columns 0..1024 of xT (keys are consistently permuted; softmax + PV are
permutation-invariant over keys, so the output is unchanged).  This lets
one shared program serve all 8 cores without a separate xQT input.

v4 - single fused phase around the measured bottlenecks:
  * ACT exp is the steady-state pacer; ACT does ONLY exp.
  * Projections for later head-pairs, the V tiles and the first output
    projection are injected INTO the attention stream.  Per iteration
    the emission order is: S pair -> filler matmuls -> exp -> mask-mul
    -> PV pair -> filler copies + normalize dribble, so the DVE queue
    never head-of-line-blocks the mask-mul that gates the PVs.
  * The 13-op normalize chain of each pair is dribbled into the NEXT
    pair's iterations; PSUM drains first (denominator + unnormalized
    numerator copies), recip/broadcast/mul run SBUF-side.  SBUF-only
    copies and every 4th mask-mul go to GpSimd to keep DVE under its
    budget.  Custom-DVE recip only works at partition base 0; all
    tensor-op inputs must share partition offsets (mod-32 aligned).
PSUM = 8 banks: 2x[128,1024] "sp" ring (S tiles; prologue + V fillers
ride it), 1x[128,512] "fl" (K/Q + O-proj fillers), 2x ote + 1x oto.
"""

import sys

sys.path.insert(0, "/opt/trn_rl_repo")

import numpy as np
import ml_dtypes

B, T, D, H = 2, 4096, 512, 8
DH = D // H          # 64
N_CORES = 8
QPC = 1024           # query rows per core
DC = D // 128        # 4 partition chunks of the model dim
KC = T // 128        # 32 key chunks

_BUILT = {}


def _build(with_bias: bool):
    from concourse import bacc
    import concourse.mybir as mybir
    import concourse.tile as tile

    dt = mybir.dt
    AF = mybir.ActivationFunctionType

    nc = bacc.Bacc("TRN2", target_bir_lowering=False, debug=False,
                   num_devices=N_CORES)

    xT = nc.dram_tensor("xT", [128, DC, T], dt.bfloat16, kind="ExternalInput").ap()
    wq = nc.dram_tensor("wq", [128, DC, D], dt.bfloat16, kind="ExternalInput").ap()
    wk = nc.dram_tensor("wk", [128, DC, D], dt.bfloat16, kind="ExternalInput").ap()
    wv = nc.dram_tensor("wv", [128, DC, D], dt.bfloat16, kind="ExternalInput").ap()
    wo = nc.dram_tensor("wo", [128, DC, D], dt.bfloat16, kind="ExternalInput").ap()
    msk = nc.dram_tensor("msk", [128, KC, QPC], dt.bfloat16, kind="ExternalInput").ap()
    if with_bias:
        bqkd = nc.dram_tensor("bqk", [128, DC, 2], dt.float32, kind="ExternalInput").ap()
        bvo = nc.dram_tensor("bvo", [1, 2, D], dt.bfloat16, kind="ExternalInput").ap()
        ones1 = nc.dram_tensor("ones1", [1, 128], dt.bfloat16, kind="ExternalInput").ap()
    out = nc.dram_tensor("out", [QPC, D], dt.float32, kind="ExternalOutput").ap()

    with tile.TileContext(nc) as tc:
        with (
            tc.tile_pool(name="persist", bufs=1) as pp,
            tc.tile_pool(name="psR", bufs=2, space="PSUM") as psR,
            tc.tile_pool(name="psO", bufs=1, space="PSUM") as psO,
            tc.tile_pool(name="pB", bufs=3) as pB,
            tc.tile_pool(name="pN", bufs=1) as pN,
            tc.tile_pool(name="pC", bufs=1) as pC,
        ):
            wq_sb = pp.tile([128, DC, D], dt.bfloat16, tag="wq")
            wk_sb = pp.tile([128, DC, D], dt.bfloat16, tag="wk")
            wv_sb = pp.tile([128, DC, D], dt.bfloat16, tag="wv")
            wo_sb = pp.tile([128, DC, D], dt.bfloat16, tag="wo")
            msk_sb = pp.tile([128, KC, QPC], dt.bfloat16, tag="msk")
            xt_sb = pp.tile([128, DC, T], dt.bfloat16, tag="xt")
            kt_f = [pp.tile([128, T], dt.bfloat16, tag=f"kt{fo}", name=f"kt{fo}")
                    for fo in range(DC)]
            qt_f = [pp.tile([128, QPC], dt.bfloat16, tag=f"qt{fo}", name=f"qt{fo}")
                    for fo in range(DC)]
            ot_f = [pp.tile([128, QPC], dt.bfloat16, tag=f"ot{fo}", name=f"ot{fo}")
                    for fo in range(DC)]
            v_c = [pp.tile([128, H, DH + 1], dt.bfloat16, tag=f"v{kc}", name=f"v{kc}")
                   for kc in range(KC)]
            recA = pp.tile([128, 512], dt.float32, tag="recA")
            den0 = pp.tile([1, 512], dt.float32, tag="den0")

            if with_bias:
                bqk_sb = pp.tile([128, DC, 2], dt.float32, tag="bqk")
                bvo_sb = pp.tile([1, 2, D], dt.bfloat16, tag="bvo")
                ones1_sb = pp.tile([1, 128], dt.bfloat16, tag="ones1")

            # ---- input DMAs (issue order = arrival order on the queue).
            # wk+wq first so prologue projections start at the first xt piece.
            nc.sync.dma_start(wk_sb[:], wk[:])
            nc.sync.dma_start(wq_sb[:], wq[:])
            for c4 in range(4):
                ts = slice(c4 * 1024, (c4 + 1) * 1024)
                nc.sync.dma_start(xt_sb[:, :, ts], xT[:, :, ts])
            for lo, hi in ((0, 2), (2, 4)):
                nc.sync.dma_start(msk_sb[:, lo:hi, :], msk[:, lo:hi, :])
            nc.sync.dma_start(wv_sb[:], wv[:])
            for lo, hi in ((4, 8), (8, 12), (12, 16)):
                nc.sync.dma_start(msk_sb[:, lo:hi, :], msk[:, lo:hi, :])
            nc.sync.dma_start(wo_sb[:], wo[:])
            if with_bias:
                nc.sync.dma_start(bqk_sb[:], bqkd[:])
                nc.sync.dma_start(bvo_sb[:], bvo[:])
                nc.sync.dma_start(ones1_sb[:], ones1[:])
            for lo, hi in ((16, 20), (20, 24), (24, 28), (28, 32)):
                nc.sync.dma_start(msk_sb[:, lo:hi, :], msk[:, lo:hi, :])

            # ones columns of V_aug (denominator accumulators)
            nc.vector.memset(recA[:], 1.0)
            for kc in range(KC):
                nc.gpsimd.memset(v_c[kc][:, :, DH:DH + 1], 1.0)

            # ------------- work units (mm part / copy part split) -------
            def kq_unit(w_sb, fo, tok0, out_ap, bi, tag):
                """[128,512] K^T/Q^T chunk -> psum; returns the copy part."""
                ps = psR.tile([128, 512], dt.float32, tag=tag,
                              bufs=(1 if tag == "fl" else None))
                for dc in range(DC):
                    nc.tensor.matmul(
                        ps[:],
                        w_sb[:, dc, fo * 128:(fo + 1) * 128],
                        xt_sb[:, dc, tok0:tok0 + 512],
                        start=(dc == 0), stop=(dc == DC - 1),
                    )

                def fin():
                    if with_bias:
                        nc.vector.tensor_scalar(
                            out_ap, ps[:], bqk_sb[:, fo, bi:bi + 1], 0.0,
                            mybir.AluOpType.add, mybir.AluOpType.bypass,
                        )
                    else:
                        nc.vector.tensor_copy(out_ap, ps[:])
                return fin

            def v_unit(tt, on_act=False):
                """V chunk tt -> psum; returns the copy part."""
                ps = psR.tile([128, 512], dt.float32, tag="sp")
                for dc in range(DC):
                    nc.tensor.matmul(
                        ps[:],
                        xt_sb[:, dc, tt * 128:(tt + 1) * 128],
                        wv_sb[:, dc, :],
                        start=(dc == 0),
                        stop=(not with_bias and dc == DC - 1),
                    )
                if with_bias:
                    nc.tensor.matmul(ps[:], ones1_sb[:], bvo_sb[:, 0, :],
                                     start=False, stop=True)

                def fin():
                    src = ps[:].rearrange("p (h f) -> p h f", h=H)
                    if on_act:
                        nc.scalar.copy(v_c[tt][:, :, 0:DH], src)
                    else:
                        nc.vector.tensor_copy(v_c[tt][:, :, 0:DH], src)
                return fin

            def oproj_unit(tt, tag="fl", staging=None):
                """Output projection for token chunk tt (128 rows)."""
                ps = psR.tile([128, 512], dt.float32, tag=tag,
                              bufs=(1 if tag == "fl" else None))
                for dc in range(DC):
                    nc.tensor.matmul(
                        ps[:],
                        ot_f[dc][:, tt * 128:(tt + 1) * 128],
                        wo_sb[:, dc, :],
                        start=(dc == 0),
                        stop=(with_bias is False and dc == DC - 1))
                if with_bias:
                    nc.tensor.matmul(ps[:], ones1_sb[:],
                                     bvo_sb[:, 1, :], start=False, stop=True)

                def fin():
                    os = staging if staging is not None else \
                        pC.tile([128, 512], dt.float32, tag="os")
                    nc.vector.tensor_copy(os[:, 0:512], ps[:])
                    nc.sync.dma_start(out[tt * 128:(tt + 1) * 128, :],
                                      os[:, 0:512])
                return fin

            # ---------------- prologue: K+Q for fo0 --------------------
            # Q first (needs only the first xt piece); copies on idle DVE.
            pro = []

            def pro_emit(fin):
                pro.append(fin)
                if len(pro) >= 2:
                    pro.pop(0)()

            for nb in range(2):
                pro_emit(kq_unit(wq_sb, 0, nb * 512,
                                 qt_f[0][:, nb * 512:(nb + 1) * 512], 0, "sp"))
            for nb in range(8):
                pro_emit(kq_unit(wk_sb, 0, nb * 512,
                                 kt_f[0][:, nb * 512:(nb + 1) * 512], 1, "sp"))
            pro_emit(v_unit(0))
            for f in pro:
                f()

            # pair order: (jq0,pr0),(jq1,pr0),(jq0,pr1),(jq1,pr1), ...
            pairs = [(jq, pr) for pr in range(H // 2) for jq in range(2)]

            def kq_group(fo):
                units = []
                for nb in range(8):
                    units.append(lambda fo=fo, nb=nb: kq_unit(
                        wk_sb, fo, nb * 512,
                        kt_f[fo][:, nb * 512:(nb + 1) * 512], 1, "fl"))
                for nb in range(2):
                    units.append(lambda fo=fo, nb=nb: kq_unit(
                        wq_sb, fo, nb * 512,
                        qt_f[fo][:, nb * 512:(nb + 1) * 512], 0, "fl"))
                return units

            g1, g2, g3 = kq_group(1), kq_group(2), kq_group(3)
            fillers = {
                0: ([(lambda tt=tt: v_unit(tt, on_act=(tt % 2 == 1)))
                     for tt in range(1, KC)] + g1[:5]),
                1: g1[5:],
                2: g2[:5], 3: g2[5:],
                4: g3[:5], 5: g3[5:],
                7: [lambda tt=tt: oproj_unit(tt) for tt in range(4)],
            }

            def normalize_ops(pr, qs, ot_e, ot_o):
                """13 ops; PSUM drains in the first five, the rest is SBUF."""
                oslc_e = ot_f[pr][0:64, qs]
                oslc_o = ot_f[pr][64:128, qs]
                bcs = pN.tile([128, 512], dt.float32, tag="bcs")
                nc.vector.tensor_copy(den0[:], ot_o[DH:DH + 1, :])
                nc.vector.tensor_copy(oslc_o, ot_o[0:DH, :])
                return [
                    lambda: nc.vector.reciprocal_approx_fast(recA[0:1, :], den0[:]),
                    lambda: nc.vector.tensor_copy(den0[:], ot_e[DH:DH + 1, :]),
                    lambda: nc.vector.tensor_copy(oslc_e, ot_e[0:DH, :]),
                    lambda: nc.vector.tensor_copy(recA[64:65, :], recA[0:1, :]),
                    lambda: nc.vector.tensor_copy(recA[96:97, :], recA[0:1, :]),
                    lambda: nc.vector.stream_shuffle(bcs[64:128, :],
                                                     recA[64:128, :], [0] * 32),
                    lambda: nc.vector.tensor_mul(oslc_o, oslc_o, bcs[64:128, :]),
                    lambda: nc.vector.reciprocal_approx_fast(recA[0:1, :], den0[:]),
                    lambda: nc.vector.tensor_copy(recA[32:33, :], recA[0:1, :]),
                    lambda: nc.vector.stream_shuffle(bcs[0:64, :],
                                                     recA[0:64, :], [0] * 32),
                    lambda: nc.vector.tensor_mul(oslc_e, oslc_e, bcs[0:64, :]),
                ]

            dribble = []          # previous pair's normalize, 2 ops/iter
            for pi, (jq, pr) in enumerate(pairs):
                qs = slice(jq * 512, (jq + 1) * 512)
                pend = list(fillers.get(pi, ()))
                inject_at = {}
                if pend:
                    # pair 7's O-proj fillers read ot_f written by pair 6's
                    # normalize, which dribbles into iters 0..5 here - delay.
                    start = 8 if pi == 7 else 0
                    step = (KC - start) / len(pend)
                    for u, unit in enumerate(pend):
                        inject_at.setdefault(start + int(u * step), []).append(unit)

                ot_e = psO.tile([DH + 1, 512], dt.float32, tag="ote", bufs=2)
                ot_o = psO.tile([DH + 1, 512], dt.float32, tag="oto", bufs=1)
                for kc in range(KC):
                    ks = slice(kc * 128, (kc + 1) * 128)
                    sp = psR.tile([128, 1024], dt.float32, tag="sp")
                    nc.tensor.matmul(sp[:, 0:512], kt_f[pr][0:64, ks],
                                     qt_f[pr][0:64, qs], start=True, stop=True)
                    nc.tensor.matmul(sp[:, 512:1024], kt_f[pr][64:128, ks],
                                     qt_f[pr][64:128, qs], start=True, stop=True)
                    finishers = [unit() for unit in inject_at.get(kc, ())]
                    p_sb = pB.tile([128, 1024], dt.bfloat16, tag="p")
                    nc.scalar.activation(p_sb[:], sp[:], AF.Exp, scale=0.125)
                    pv = p_sb[:].rearrange("p (a b) -> p a b", a=2)
                    mk = msk_sb[:, kc, qs][:, None, :].to_broadcast((128, 2, 512))
                    nc.vector.tensor_mul(pv, pv, mk)
                    nc.tensor.matmul(ot_e[:], v_c[kc][:, 2 * pr, :],
                                     p_sb[:, 0:512],
                                     start=(kc == 0), stop=(kc == KC - 1))
                    nc.tensor.matmul(ot_o[:], v_c[kc][:, 2 * pr + 1, :],
                                     p_sb[:, 512:1024],
                                     start=(kc == 0), stop=(kc == KC - 1))
                    for f in finishers:
                        f()
                    ndrib = 3 if kc == 0 else 2
                    while dribble and ndrib > 0:
                        dribble.pop(0)()
                        ndrib -= 1

                for op in dribble:    # leftovers (shouldn't happen)
                    op()
                dribble = normalize_ops(pr, qs, ot_e, ot_o)

            for op in dribble:        # last pair's normalize
                op()

            # tail: output projection of the last query block, pipelined on
            # the now-free "sp" ring with alternating staging tiles.
            bcs_stage = pN.tile([128, 512], dt.float32, tag="bcs")
            fins = []
            for i, tt in enumerate(range(4, 8)):
                fins.append(oproj_unit(
                    tt, tag="sp",
                    staging=(bcs_stage if i % 2 else None)))
                if i >= 1:
                    fins.pop(0)()
            for f in fins:
                f()

    nc.compile()
    return nc


def _get_nc(with_bias: bool):
    if with_bias not in _BUILT:
        _BUILT[with_bias] = _build(with_bias)
    return _BUILT[with_bias]


def _prep_inputs(x, Wq, bq, Wk, bk, Wv, bv, Wo, bo, mask, with_bias):
    bf16 = ml_dtypes.bfloat16

    shared = {}
    for name, W in (("wq", Wq), ("wk", Wk), ("wv", Wv), ("wo", Wo)):
        shared[name] = np.ascontiguousarray(
            np.asarray(W, np.float32).astype(bf16)
            .reshape(DC, 128, D).transpose(1, 0, 2))
    if with_bias:
        shared["bqk"] = np.ascontiguousarray(np.stack(
            [np.asarray(bq, np.float32).reshape(DC, 128).T,
             np.asarray(bk, np.float32).reshape(DC, 128).T], axis=-1))
        shared["bvo"] = np.ascontiguousarray(np.stack(
            [np.asarray(bv, np.float32), np.asarray(bo, np.float32)]
        ).astype(bf16).reshape(1, 2, D))
        shared["ones1"] = np.ones((1, 128), np.float32).astype(bf16)

    maskT = np.asarray(mask).reshape(T, T).T          # (k, q)
    m01T = maskT.astype(np.float32)

    in_maps = []
    for c in range(N_CORES):
        b, q0 = c // 4, (c % 4) * QPC
        # Roll the token axis so this core's query block sits at columns
        # 0..QPC; keys are consistently permuted (softmax + PV are
        # permutation-invariant over keys); the mask key axis rolls too.
        xTb = np.asarray(x[b], np.float32).T.astype(bf16)     # (D, T)
        xTr = np.roll(xTb, -q0, axis=1)
        m = dict(shared)
        m["xT"] = np.ascontiguousarray(
            xTr.reshape(DC, 128, T).transpose(1, 0, 2))
        mr = np.roll(m01T[:, q0:q0 + QPC], -q0, axis=0)       # (k rolled, q)
        m["msk"] = np.ascontiguousarray(
            mr.reshape(KC, 128, QPC).transpose(1, 0, 2)).astype(bf16)
        in_maps.append(m)
    return in_maps


def _run(inputs, trace=False):
    from concourse.bass_utils import run_bass_kernel_spmd

    with_bias = any(
        float(np.abs(np.asarray(inputs[k], np.float32)).max()) != 0.0
        for k in ("bq", "bk", "bv", "bo"))
    nc = _get_nc(with_bias)
    in_maps = _prep_inputs(
        inputs["x"], inputs["Wq"], inputs["bq"], inputs["Wk"], inputs["bk"],
        inputs["Wv"], inputs["bv"], inputs["Wo"], inputs["bo"],
        inputs["mask"], with_bias)
    res = run_bass_kernel_spmd(nc, in_maps, list(range(N_CORES)), trace=trace)
    O = np.empty((B, T, D), np.float32)
    for c in range(N_CORES):
        b, q0 = c // 4, (c % 4) * QPC
        O[b, q0:q0 + QPC, :] = res.results[c]["out"]
    return O, res


def kernel(**inputs) -> np.ndarray:
    out, _ = _run(inputs, trace=False)
    return out


# revision 16
# speedup vs baseline: 1.4405x; 1.0169x over previous
"""Multi-head attention (B=2, T=4096, D=512, H=8) on 8 TRN2 NeuronCores.

Sharding: core c handles batch c//4 and query rows (c%4)*1024..+1024.
Heads stay together on a core; K/V are recomputed per core (no comm).
The host ROLLS the token axis per core so the core's query block sits at
columns 0..1024 of xT (keys are consistently permuted; softmax + PV are
permutation-invariant over keys, so the output is unchanged).  This lets
one shared program serve all 8 cores without a separate xQT input.

v4 - single fused phase around the measured bottlenecks:
  * ACT exp is the steady-state pacer; ACT does ONLY exp.
  * Projections for later head-pairs, the V tiles and the first output
    projection are injected INTO the attention stream.  Per iteration
    the emission order is: S pair -> filler matmuls -> exp -> mask-mul
    -> PV pair -> filler copies + normalize dribble, so the DVE queue
    never head-of-line-blocks the mask-mul that gates the PVs.
  * The 13-op normalize chain of each pair is dribbled into the NEXT
    pair's iterations; PSUM drains first (denominator + unnormalized
    numerator copies), recip/broadcast/mul run SBUF-side.  SBUF-only
    copies and every 4th mask-mul go to GpSimd to keep DVE under its
    budget.  Custom-DVE recip only works at partition base 0; all
    tensor-op inputs must share partition offsets (mod-32 aligned).
PSUM = 8 banks: 2x[128,1024] "sp" ring (S tiles; prologue + V fillers
ride it), 1x[128,512] "fl" (K/Q + O-proj fillers), 2x ote + 1x oto.
"""

import sys

sys.path.insert(0, "/opt/trn_rl_repo")

import numpy as np
import ml_dtypes

B, T, D, H = 2, 4096, 512, 8
DH = D // H          # 64
N_CORES = 8
QPC = 1024           # query rows per core
DC = D // 128        # 4 partition chunks of the model dim
KC = T // 128        # 32 key chunks

_BUILT = {}


def _build(with_bias: bool):
    from concourse import bacc
    import concourse.mybir as mybir
    import concourse.tile as tile

    dt = mybir.dt
    AF = mybir.ActivationFunctionType

    nc = bacc.Bacc("TRN2", target_bir_lowering=False, debug=False,
                   num_devices=N_CORES)

    xT = nc.dram_tensor("xT", [128, DC, T], dt.bfloat16, kind="ExternalInput").ap()
    wq = nc.dram_tensor("wq", [128, DC, D], dt.bfloat16, kind="ExternalInput").ap()
    wk = nc.dram_tensor("wk", [128, DC, D], dt.bfloat16, kind="ExternalInput").ap()
    wv = nc.dram_tensor("wv", [128, DC, D], dt.bfloat16, kind="ExternalInput").ap()
    wo = nc.dram_tensor("wo", [128, DC, D], dt.bfloat16, kind="ExternalInput").ap()
    msk = nc.dram_tensor("msk", [128, KC, QPC], dt.bfloat16, kind="ExternalInput").ap()
    if with_bias:
        bqkd = nc.dram_tensor("bqk", [128, DC, 2], dt.float32, kind="ExternalInput").ap()
        bvo = nc.dram_tensor("bvo", [1, 2, D], dt.bfloat16, kind="ExternalInput").ap()
        ones1 = nc.dram_tensor("ones1", [1, 128], dt.bfloat16, kind="ExternalInput").ap()
    out = nc.dram_tensor("out", [QPC, D], dt.float32, kind="ExternalOutput").ap()

    with tile.TileContext(nc) as tc:
        with (
            tc.tile_pool(name="persist", bufs=1) as pp,
            tc.tile_pool(name="psR", bufs=2, space="PSUM") as psR,
            tc.tile_pool(name="psO", bufs=1, space="PSUM") as psO,
            tc.tile_pool(name="pB", bufs=3) as pB,
            tc.tile_pool(name="pN", bufs=1) as pN,
            tc.tile_pool(name="pC", bufs=1) as pC,
        ):
            wq_sb = pp.tile([128, DC, D], dt.bfloat16, tag="wq")
            wk_sb = pp.tile([128, DC, D], dt.bfloat16, tag="wk")
            wv_sb = pp.tile([128, DC, D], dt.bfloat16, tag="wv")
            wo_sb = pp.tile([128, DC, D], dt.bfloat16, tag="wo")
            msk_sb = pp.tile([128, KC, QPC], dt.bfloat16, tag="msk")
            xt_sb = pp.tile([128, DC, T], dt.bfloat16, tag="xt")
            kt_f = [pp.tile([128, T], dt.bfloat16, tag=f"kt{fo}", name=f"kt{fo}")
                    for fo in range(DC)]
            qt_f = [pp.tile([128, QPC], dt.bfloat16, tag=f"qt{fo}", name=f"qt{fo}")
                    for fo in range(DC)]
            ot_f = [pp.tile([128, QPC], dt.bfloat16, tag=f"ot{fo}", name=f"ot{fo}")
                    for fo in range(DC)]
            v_c = [pp.tile([128, H, DH + 1], dt.bfloat16, tag=f"v{kc}", name=f"v{kc}")
                   for kc in range(KC)]
            recA = pp.tile([128, 512], dt.float32, tag="recA")
            den0 = pp.tile([1, 512], dt.float32, tag="den0")

            if with_bias:
                bqk_sb = pp.tile([128, DC, 2], dt.float32, tag="bqk")
                bvo_sb = pp.tile([1, 2, D], dt.bfloat16, tag="bvo")
                ones1_sb = pp.tile([1, 128], dt.bfloat16, tag="ones1")

            # ---- input DMAs (issue order = arrival order on the queue).
            # wk+wq first so prologue projections start at the first xt piece.
            nc.sync.dma_start(wk_sb[:], wk[:])
            nc.sync.dma_start(wq_sb[:], wq[:])
            for c4 in range(2):
                ts = slice(c4 * 1024, (c4 + 1) * 1024)
                nc.sync.dma_start(xt_sb[:, :, ts], xT[:, :, ts])
            nc.sync.dma_start(msk_sb[:, 0:2, :], msk[:, 0:2, :])
            nc.sync.dma_start(wv_sb[:], wv[:])
            nc.sync.dma_start(msk_sb[:, 2:4, :], msk[:, 2:4, :])
            for c4 in range(2, 4):
                ts = slice(c4 * 1024, (c4 + 1) * 1024)
                nc.sync.dma_start(xt_sb[:, :, ts], xT[:, :, ts])
            for lo, hi in ((4, 8), (8, 12), (12, 16)):
                nc.sync.dma_start(msk_sb[:, lo:hi, :], msk[:, lo:hi, :])
            nc.sync.dma_start(wo_sb[:], wo[:])
            if with_bias:
                nc.sync.dma_start(bqk_sb[:], bqkd[:])
                nc.sync.dma_start(bvo_sb[:], bvo[:])
                nc.sync.dma_start(ones1_sb[:], ones1[:])
            for lo, hi in ((16, 20), (20, 24), (24, 28), (28, 32)):
                nc.sync.dma_start(msk_sb[:, lo:hi, :], msk[:, lo:hi, :])

            # ones columns of V_aug (denominator accumulators)
            nc.vector.memset(recA[:], 1.0)
            for kc in range(KC):
                nc.gpsimd.memset(v_c[kc][:, :, DH:DH + 1], 1.0)

            # ------------- work units (mm part / copy part split) -------
            def kq_unit(w_sb, fo, tok0, out_ap, bi, tag):
                """[128,512] K^T/Q^T chunk -> psum; returns the copy part."""
                ps = psR.tile([128, 512], dt.float32, tag=tag,
                              bufs=(1 if tag == "fl" else None))
                for dc in range(DC):
                    nc.tensor.matmul(
                        ps[:],
                        w_sb[:, dc, fo * 128:(fo + 1) * 128],
                        xt_sb[:, dc, tok0:tok0 + 512],
                        start=(dc == 0), stop=(dc == DC - 1),
                    )

                def fin():
                    if with_bias:
                        nc.vector.tensor_scalar(
                            out_ap, ps[:], bqk_sb[:, fo, bi:bi + 1], 0.0,
                            mybir.AluOpType.add, mybir.AluOpType.bypass,
                        )
                    else:
                        nc.vector.tensor_copy(out_ap, ps[:])
                return fin

            def v_unit(tt, on_act=False):
                """V chunk tt -> psum; returns the copy part."""
                ps = psR.tile([128, 512], dt.float32, tag="sp")
                for dc in range(DC):
                    nc.tensor.matmul(
                        ps[:],
                        xt_sb[:, dc, tt * 128:(tt + 1) * 128],
                        wv_sb[:, dc, :],
                        start=(dc == 0),
                        stop=(not with_bias and dc == DC - 1),
                    )
                if with_bias:
                    nc.tensor.matmul(ps[:], ones1_sb[:], bvo_sb[:, 0, :],
                                     start=False, stop=True)

                def fin():
                    src = ps[:].rearrange("p (h f) -> p h f", h=H)
                    if on_act:
                        nc.scalar.copy(v_c[tt][:, :, 0:DH], src)
                    else:
                        nc.vector.tensor_copy(v_c[tt][:, :, 0:DH], src)
                return fin

            def oproj_unit(tt, tag="fl", staging=None):
                """Output projection for token chunk tt (128 rows)."""
                ps = psR.tile([128, 512], dt.float32, tag=tag,
                              bufs=(1 if tag == "fl" else None))
                for dc in range(DC):
                    nc.tensor.matmul(
                        ps[:],
                        ot_f[dc][:, tt * 128:(tt + 1) * 128],
                        wo_sb[:, dc, :],
                        start=(dc == 0),
                        stop=(with_bias is False and dc == DC - 1))
                if with_bias:
                    nc.tensor.matmul(ps[:], ones1_sb[:],
                                     bvo_sb[:, 1, :], start=False, stop=True)

                def fin():
                    os = staging if staging is not None else \
                        pC.tile([128, 512], dt.float32, tag="os")
                    nc.vector.tensor_copy(os[:, 0:512], ps[:])
                    nc.sync.dma_start(out[tt * 128:(tt + 1) * 128, :],
                                      os[:, 0:512])
                return fin

            # ---------------- prologue: K+Q for fo0 --------------------
            # Q first (needs only the first xt piece); copies on idle DVE.
            pro = []

            def pro_emit(fin):
                pro.append(fin)
                if len(pro) >= 2:
                    pro.pop(0)()

            for nb in range(2):
                pro_emit(kq_unit(wq_sb, 0, nb * 512,
                                 qt_f[0][:, nb * 512:(nb + 1) * 512], 0, "sp"))
            for nb in range(4):
                pro_emit(kq_unit(wk_sb, 0, nb * 512,
                                 kt_f[0][:, nb * 512:(nb + 1) * 512], 1, "sp"))
            for f in pro:
                f()

            # pair order: (jq0,pr0),(jq1,pr0),(jq0,pr1),(jq1,pr1), ...
            pairs = [(jq, pr) for pr in range(H // 2) for jq in range(2)]

            def kq_group(fo):
                units = []
                for nb in range(8):
                    units.append(lambda fo=fo, nb=nb: kq_unit(
                        wk_sb, fo, nb * 512,
                        kt_f[fo][:, nb * 512:(nb + 1) * 512], 1, "fl"))
                for nb in range(2):
                    units.append(lambda fo=fo, nb=nb: kq_unit(
                        wq_sb, fo, nb * 512,
                        qt_f[fo][:, nb * 512:(nb + 1) * 512], 0, "fl"))
                return units

            g1, g2, g3 = kq_group(1), kq_group(2), kq_group(3)
            k47 = [lambda nb=nb: kq_unit(
                wk_sb, 0, nb * 512,
                kt_f[0][:, nb * 512:(nb + 1) * 512], 1, "fl")
                for nb in range(4, 8)]
            fillers = {
                1: g1[5:],
                2: g2[:5], 3: g2[5:],
                4: g3[:5], 5: g3[5:],
                7: [lambda tt=tt: oproj_unit(tt) for tt in range(4)],
            }
            # pair 0: V chunks chase their PVs (v(tt) at iter <= tt); the
            # deferred K chunks 4-7 land before the S iters that need them
            # (xt pieces 2-3 arrive ~iter 4-7); kq(fo1) afterwards.
            p0 = {}
            for u in range(1, KC):
                p0.setdefault(min(u - 1, int(u * 0.8)), []).append(
                    lambda tt=u: v_unit(tt, on_act=(tt % 2 == 1)))
            for j, unit in enumerate(k47):
                p0.setdefault(6 + 2 * j, []).append(unit)
            for j, unit in enumerate(g1[:5]):
                p0.setdefault(14 + 2 * j, []).append(unit)

            def normalize_ops(pr, qs, ot_e, ot_o):
                """13 ops; PSUM drains in the first five, the rest is SBUF."""
                oslc_e = ot_f[pr][0:64, qs]
                oslc_o = ot_f[pr][64:128, qs]
                bcs = pN.tile([128, 512], dt.float32, tag="bcs")
                nc.vector.tensor_copy(den0[:], ot_o[DH:DH + 1, :])
                nc.vector.tensor_copy(oslc_o, ot_o[0:DH, :])
                return [
                    lambda: nc.vector.reciprocal_approx_fast(recA[0:1, :], den0[:]),
                    lambda: nc.vector.tensor_copy(den0[:], ot_e[DH:DH + 1, :]),
                    lambda: nc.vector.tensor_copy(oslc_e, ot_e[0:DH, :]),
                    lambda: nc.gpsimd.tensor_copy(recA[64:65, :], recA[0:1, :]),
                    lambda: nc.gpsimd.tensor_copy(recA[96:97, :], recA[0:1, :]),
                    lambda: nc.vector.stream_shuffle(bcs[64:128, :],
                                                     recA[64:128, :], [0] * 32),
                    lambda: nc.vector.tensor_mul(oslc_o, oslc_o, bcs[64:128, :]),
                    lambda: nc.vector.reciprocal_approx_fast(recA[0:1, :], den0[:]),
                    lambda: nc.gpsimd.tensor_copy(recA[32:33, :], recA[0:1, :]),
                    lambda: nc.vector.stream_shuffle(bcs[0:64, :],
                                                     recA[0:64, :], [0] * 32),
                    lambda: nc.vector.tensor_mul(oslc_e, oslc_e, bcs[0:64, :]),
                ]

            dribble = []          # previous pair's normalize, 2 ops/iter
            for pi, (jq, pr) in enumerate(pairs):
                qs = slice(jq * 512, (jq + 1) * 512)
                if pi == 0:
                    inject_at = p0
                    v_unit(0)()  # fully before the kc loop: PV(0) needs it
                else:
                    pend = list(fillers.get(pi, ()))
                    inject_at = {}
                    if pend:
                        # pair 7's O-proj fillers read ot_f written by pair
                        # 6's normalize, which dribbles into iters 0..18
                        # here - delay past it.
                        start = 20 if pi == 7 else 0
                        step = (KC - start) / len(pend)
                        for u, unit in enumerate(pend):
                            inject_at.setdefault(
                                start + int(u * step), []).append(unit)

                ot_e = psO.tile([DH + 1, 512], dt.float32, tag="ote", bufs=2)
                ot_o = psO.tile([DH + 1, 512], dt.float32, tag="oto", bufs=1)
                for kc in range(KC):
                    ks = slice(kc * 128, (kc + 1) * 128)
                    sp = psR.tile([128, 1024], dt.float32, tag="sp")
                    nc.tensor.matmul(sp[:, 0:512], kt_f[pr][0:64, ks],
                                     qt_f[pr][0:64, qs], start=True, stop=True)
                    nc.tensor.matmul(sp[:, 512:1024], kt_f[pr][64:128, ks],
                                     qt_f[pr][64:128, qs], start=True, stop=True)
                    finishers = [unit() for unit in inject_at.get(kc, ())]
                    p_sb = pB.tile([128, 1024], dt.bfloat16, tag="p")
                    nc.scalar.activation(p_sb[:], sp[:], AF.Exp, scale=0.125)
                    pv = p_sb[:].rearrange("p (a b) -> p a b", a=2)
                    mk = msk_sb[:, kc, qs][:, None, :].to_broadcast((128, 2, 512))
                    nc.vector.tensor_mul(pv, pv, mk)
                    nc.tensor.matmul(ot_e[:], v_c[kc][:, 2 * pr, :],
                                     p_sb[:, 0:512],
                                     start=(kc == 0), stop=(kc == KC - 1))
                    nc.tensor.matmul(ot_o[:], v_c[kc][:, 2 * pr + 1, :],
                                     p_sb[:, 512:1024],
                                     start=(kc == 0), stop=(kc == KC - 1))
                    for f in finishers:
                        f()
                    ndrib = 2 if kc == 0 else (1 if kc % 2 == 0 else 0)
                    while dribble and ndrib > 0:
                        dribble.pop(0)()
                        ndrib -= 1

                for op in dribble:    # leftovers (shouldn't happen)
                    op()
                dribble = normalize_ops(pr, qs, ot_e, ot_o)

            for op in dribble:        # last pair's normalize
                op()

            # tail: output projection of the last query block, pipelined on
            # the now-free "sp" ring with alternating staging tiles.
            bcs_stage = pN.tile([128, 512], dt.float32, tag="bcs")
            fins = []
            for i, tt in enumerate(range(4, 8)):
                fins.append(oproj_unit(
                    tt, tag="sp",
                    staging=(bcs_stage if i % 2 else None)))
                if i >= 1:
                    fins.pop(0)()
            for f in fins:
                f()

    nc.compile()
    return nc


def _get_nc(with_bias: bool):
    if with_bias not in _BUILT:
        _BUILT[with_bias] = _build(with_bias)
    return _BUILT[with_bias]


def _prep_inputs(x, Wq, bq, Wk, bk, Wv, bv, Wo, bo, mask, with_bias):
    bf16 = ml_dtypes.bfloat16

    shared = {}
    for name, W in (("wq", Wq), ("wk", Wk), ("wv", Wv), ("wo", Wo)):
        shared[name] = np.ascontiguousarray(
            np.asarray(W, np.float32).astype(bf16)
            .reshape(DC, 128, D).transpose(1, 0, 2))
    if with_bias:
        shared["bqk"] = np.ascontiguousarray(np.stack(
            [np.asarray(bq, np.float32).reshape(DC, 128).T,
             np.asarray(bk, np.float32).reshape(DC, 128).T], axis=-1))
        shared["bvo"] = np.ascontiguousarray(np.stack(
            [np.asarray(bv, np.float32), np.asarray(bo, np.float32)]
        ).astype(bf16).reshape(1, 2, D))
        shared["ones1"] = np.ones((1, 128), np.float32).astype(bf16)

    maskT = np.asarray(mask).reshape(T, T).T          # (k, q)
    m01T = maskT.astype(np.float32)

    in_maps = []
    for c in range(N_CORES):
        b, q0 = c // 4, (c % 4) * QPC
        # Roll the token axis so this core's query block sits at columns
        # 0..QPC; keys are consistently permuted (softmax + PV are
        # permutation-invariant over keys); the mask key axis rolls too.
        xTb = np.asarray(x[b], np.float32).T.astype(bf16)     # (D, T)
        xTr = np.roll(xTb, -q0, axis=1)
        m = dict(shared)
        m["xT"] = np.ascontiguousarray(
            xTr.reshape(DC, 128, T).transpose(1, 0, 2))
        mr = np.roll(m01T[:, q0:q0 + QPC], -q0, axis=0)       # (k rolled, q)
        m["msk"] = np.ascontiguousarray(
            mr.reshape(KC, 128, QPC).transpose(1, 0, 2)).astype(bf16)
        in_maps.append(m)
    return in_maps


def _run(inputs, trace=False):
    from concourse.bass_utils import run_bass_kernel_spmd

    with_bias = any(
        float(np.abs(np.asarray(inputs[k], np.float32)).max()) != 0.0
        for k in ("bq", "bk", "bv", "bo"))
    nc = _get_nc(with_bias)
    in_maps = _prep_inputs(
        inputs["x"], inputs["Wq"], inputs["bq"], inputs["Wk"], inputs["bk"],
        inputs["Wv"], inputs["bv"], inputs["Wo"], inputs["bo"],
        inputs["mask"], with_bias)
    res = run_bass_kernel_spmd(nc, in_maps, list(range(N_CORES)), trace=trace)
    O = np.empty((B, T, D), np.float32)
    for c in range(N_CORES):
        b, q0 = c // 4, (c % 4) * QPC
        O[b, q0:q0 + QPC, :] = res.results[c]["out"]
    return O, res


def kernel(**inputs) -> np.ndarray:
    out, _ = _run(inputs, trace=False)
    return out


# revision 18
# speedup vs baseline: 1.4655x; 1.0173x over previous
"""Multi-head attention (B=2, T=4096, D=512, H=8) on 8 TRN2 NeuronCores.

Sharding: core c handles batch c//4 and query rows (c%4)*1024..+1024.
Heads stay together on a core; K/V are recomputed per core (no comm).
The host ROLLS the token axis per core so the core's query block sits at
columns 0..1024 of xT (keys are consistently permuted; softmax + PV are
permutation-invariant over keys, so the output is unchanged).  This lets
one shared program serve all 8 cores without a separate xQT input.

v4 - single fused phase around the measured bottlenecks:
  * ACT exp is the steady-state pacer; ACT does ONLY exp.
  * Projections for later head-pairs, the V tiles and the first output
    projection are injected INTO the attention stream.  Per iteration
    the emission order is: S pair -> filler matmuls -> exp -> mask-mul
    -> PV pair -> filler copies + normalize dribble, so the DVE queue
    never head-of-line-blocks the mask-mul that gates the PVs.
  * The 13-op normalize chain of each pair is dribbled into the NEXT
    pair's iterations; PSUM drains first (denominator + unnormalized
    numerator copies), recip/broadcast/mul run SBUF-side.  SBUF-only
    copies and every 4th mask-mul go to GpSimd to keep DVE under its
    budget.  Custom-DVE recip only works at partition base 0; all
    tensor-op inputs must share partition offsets (mod-32 aligned).
PSUM = 8 banks: 2x[128,1024] "sp" ring (S tiles; prologue + V fillers
ride it), 1x[128,512] "fl" (K/Q + O-proj fillers), 2x ote + 1x oto.
"""

import sys

sys.path.insert(0, "/opt/trn_rl_repo")

import numpy as np
import ml_dtypes

B, T, D, H = 2, 4096, 512, 8
DH = D // H          # 64
N_CORES = 8
QPC = 1024           # query rows per core
DC = D // 128        # 4 partition chunks of the model dim
KC = T // 128        # 32 key chunks

_BUILT = {}


def _build(with_bias: bool):
    from concourse import bacc
    import concourse.mybir as mybir
    import concourse.tile as tile

    dt = mybir.dt
    AF = mybir.ActivationFunctionType

    nc = bacc.Bacc("TRN2", target_bir_lowering=False, debug=False,
                   num_devices=N_CORES)

    xT = nc.dram_tensor("xT", [128, DC, T], dt.bfloat16, kind="ExternalInput").ap()
    wq = nc.dram_tensor("wq", [128, DC, D], dt.bfloat16, kind="ExternalInput").ap()
    wk = nc.dram_tensor("wk", [128, DC, D], dt.bfloat16, kind="ExternalInput").ap()
    wv = nc.dram_tensor("wv", [128, DC, D], dt.bfloat16, kind="ExternalInput").ap()
    wo = nc.dram_tensor("wo", [128, DC, D], dt.bfloat16, kind="ExternalInput").ap()
    msk = nc.dram_tensor("msk", [128, KC, QPC], dt.bfloat16, kind="ExternalInput").ap()
    if with_bias:
        bqkd = nc.dram_tensor("bqk", [128, DC, 2], dt.float32, kind="ExternalInput").ap()
        bvo = nc.dram_tensor("bvo", [1, 2, D], dt.bfloat16, kind="ExternalInput").ap()
        ones1 = nc.dram_tensor("ones1", [1, 128], dt.bfloat16, kind="ExternalInput").ap()
    out = nc.dram_tensor("out", [QPC, D], dt.float32, kind="ExternalOutput").ap()

    with tile.TileContext(nc) as tc:
        with (
            tc.tile_pool(name="persist", bufs=1) as pp,
            tc.tile_pool(name="psR", bufs=2, space="PSUM") as psR,
            tc.tile_pool(name="psO", bufs=1, space="PSUM") as psO,
            tc.tile_pool(name="pB", bufs=3) as pB,
            tc.tile_pool(name="pN", bufs=1) as pN,
            tc.tile_pool(name="pC", bufs=1) as pC,
        ):
            wq_sb = pp.tile([128, DC, D], dt.bfloat16, tag="wq")
            wk_sb = pp.tile([128, DC, D], dt.bfloat16, tag="wk")
            wv_sb = pp.tile([128, DC, D], dt.bfloat16, tag="wv")
            wo_sb = pp.tile([128, DC, D], dt.bfloat16, tag="wo")
            msk_sb = pp.tile([128, KC, QPC], dt.bfloat16, tag="msk")
            xt_sb = pp.tile([128, DC, T], dt.bfloat16, tag="xt")
            kt_f = [pp.tile([128, T], dt.bfloat16, tag=f"kt{fo}", name=f"kt{fo}")
                    for fo in range(DC)]
            qt_f = [pp.tile([128, QPC], dt.bfloat16, tag=f"qt{fo}", name=f"qt{fo}")
                    for fo in range(DC)]
            ot_f = [pp.tile([128, QPC], dt.bfloat16, tag=f"ot{fo}", name=f"ot{fo}")
                    for fo in range(DC)]
            v_c = [pp.tile([128, H, DH + 1], dt.bfloat16, tag=f"v{kc}", name=f"v{kc}")
                   for kc in range(KC)]
            recA = pp.tile([128, 512], dt.float32, tag="recA")
            den0 = pp.tile([1, 512], dt.float32, tag="den0")

            if with_bias:
                bqk_sb = pp.tile([128, DC, 2], dt.float32, tag="bqk")
                bvo_sb = pp.tile([1, 2, D], dt.bfloat16, tag="bvo")
                ones1_sb = pp.tile([1, 128], dt.bfloat16, tag="ones1")

            # ---- input DMAs (issue order = arrival order on the queue).
            # wk+wq first so prologue projections start at the first xt piece.
            nc.sync.dma_start(wk_sb[:], wk[:])
            nc.sync.dma_start(wq_sb[:], wq[:])
            for c4 in range(2):
                ts = slice(c4 * 1024, (c4 + 1) * 1024)
                nc.sync.dma_start(xt_sb[:, :, ts], xT[:, :, ts])
            nc.sync.dma_start(msk_sb[:, 0:2, :], msk[:, 0:2, :])
            nc.sync.dma_start(wv_sb[:], wv[:])
            nc.sync.dma_start(msk_sb[:, 2:4, :], msk[:, 2:4, :])
            for c4 in range(2, 4):
                ts = slice(c4 * 1024, (c4 + 1) * 1024)
                nc.sync.dma_start(xt_sb[:, :, ts], xT[:, :, ts])
            for lo, hi in ((4, 8), (8, 12), (12, 16)):
                nc.sync.dma_start(msk_sb[:, lo:hi, :], msk[:, lo:hi, :])
            nc.sync.dma_start(wo_sb[:], wo[:])
            if with_bias:
                nc.sync.dma_start(bqk_sb[:], bqkd[:])
                nc.sync.dma_start(bvo_sb[:], bvo[:])
                nc.sync.dma_start(ones1_sb[:], ones1[:])
            for lo, hi in ((16, 20), (20, 24), (24, 28), (28, 32)):
                nc.sync.dma_start(msk_sb[:, lo:hi, :], msk[:, lo:hi, :])

            # ones columns of V_aug (denominator accumulators)
            nc.vector.memset(recA[:], 1.0)
            for kc in range(KC):
                nc.gpsimd.memset(v_c[kc][:, :, DH:DH + 1], 1.0)

            # ------------- work units (mm part / copy part split) -------
            def kq_unit(w_sb, fo, tok0, out_ap, bi, tag):
                """[128,512] K^T/Q^T chunk -> psum; returns the copy part."""
                ps = psR.tile([128, 512], dt.float32, tag=tag,
                              bufs=(1 if tag == "fl" else None))
                for dc in range(DC):
                    nc.tensor.matmul(
                        ps[:],
                        w_sb[:, dc, fo * 128:(fo + 1) * 128],
                        xt_sb[:, dc, tok0:tok0 + 512],
                        start=(dc == 0), stop=(dc == DC - 1),
                    )

                def fin():
                    if with_bias:
                        nc.vector.tensor_scalar(
                            out_ap, ps[:], bqk_sb[:, fo, bi:bi + 1], 0.0,
                            mybir.AluOpType.add, mybir.AluOpType.bypass,
                        )
                    else:
                        nc.vector.tensor_copy(out_ap, ps[:])
                return fin

            def v_unit(tt, on_act=False):
                """V chunk tt -> psum; returns the copy part."""
                ps = psR.tile([128, 512], dt.float32, tag="sp")
                for dc in range(DC):
                    nc.tensor.matmul(
                        ps[:],
                        xt_sb[:, dc, tt * 128:(tt + 1) * 128],
                        wv_sb[:, dc, :],
                        start=(dc == 0),
                        stop=(not with_bias and dc == DC - 1),
                    )
                if with_bias:
                    nc.tensor.matmul(ps[:], ones1_sb[:], bvo_sb[:, 0, :],
                                     start=False, stop=True)

                def fin():
                    src = ps[:].rearrange("p (h f) -> p h f", h=H)
                    if on_act:
                        nc.scalar.copy(v_c[tt][:, :, 0:DH], src)
                    else:
                        nc.vector.tensor_copy(v_c[tt][:, :, 0:DH], src)
                return fin

            def oproj_unit(tt, tag="fl", staging=None):
                """Output projection for token chunk tt (128 rows)."""
                ps = psR.tile([128, 512], dt.float32, tag=tag,
                              bufs=(1 if tag == "fl" else None))
                for dc in range(DC):
                    nc.tensor.matmul(
                        ps[:],
                        ot_f[dc][:, tt * 128:(tt + 1) * 128],
                        wo_sb[:, dc, :],
                        start=(dc == 0),
                        stop=(with_bias is False and dc == DC - 1))
                if with_bias:
                    nc.tensor.matmul(ps[:], ones1_sb[:],
                                     bvo_sb[:, 1, :], start=False, stop=True)

                def fin():
                    os = staging if staging is not None else \
                        pC.tile([128, 512], dt.float32, tag="os")
                    nc.vector.tensor_copy(os[:, 0:512], ps[:])
                    nc.sync.dma_start(out[tt * 128:(tt + 1) * 128, :],
                                      os[:, 0:512])
                return fin

            # ---------------- prologue: K+Q for fo0 --------------------
            # Q first (needs only the first xt piece); copies on idle DVE.
            pro = []

            def pro_emit(fin):
                pro.append(fin)
                if len(pro) >= 2:
                    pro.pop(0)()

            for nb in range(2):
                pro_emit(kq_unit(wq_sb, 0, nb * 512,
                                 qt_f[0][:, nb * 512:(nb + 1) * 512], 0, "sp"))
            for nb in range(2):
                pro_emit(kq_unit(wk_sb, 0, nb * 512,
                                 kt_f[0][:, nb * 512:(nb + 1) * 512], 1, "sp"))
            for f in pro:
                f()

            # pair order: (jq0,pr0),(jq1,pr0),(jq0,pr1),(jq1,pr1), ...
            pairs = [(jq, pr) for pr in range(H // 2) for jq in range(2)]

            def kq_group(fo):
                units = []
                for nb in range(8):
                    units.append(lambda fo=fo, nb=nb: kq_unit(
                        wk_sb, fo, nb * 512,
                        kt_f[fo][:, nb * 512:(nb + 1) * 512], 1, "fl"))
                for nb in range(2):
                    units.append(lambda fo=fo, nb=nb: kq_unit(
                        wq_sb, fo, nb * 512,
                        qt_f[fo][:, nb * 512:(nb + 1) * 512], 0, "fl"))
                return units

            g1, g2, g3 = kq_group(1), kq_group(2), kq_group(3)
            k27 = [lambda nb=nb: kq_unit(
                wk_sb, 0, nb * 512,
                kt_f[0][:, nb * 512:(nb + 1) * 512], 1, "fl")
                for nb in range(2, 8)]
            fillers = {
                1: g1[5:],
                2: g2[:5], 3: g2[5:],
                4: g3[:5], 5: g3[5:],
                7: [lambda tt=tt: oproj_unit(tt) for tt in range(4)],
            }
            # pair 0: V chunks chase their PVs (v(tt) at iter <= tt); the
            # deferred K chunks 4-7 land before the S iters that need them
            # (xt pieces 2-3 arrive ~iter 4-7); kq(fo1) afterwards.
            p0 = {}
            for u in range(1, KC):
                p0.setdefault(u - 1, []).append(
                    lambda tt=u: v_unit(tt, on_act=(tt % 2 == 1)))
            for j, unit in enumerate(k27):
                p0.setdefault(1 + 2 * j, []).append(unit)
            for j, unit in enumerate(g1[:5]):
                p0.setdefault(15 + 2 * j, []).append(unit)

            def normalize_ops(pr, qs, ot_e, ot_o):
                """13 ops; PSUM drains in the first five, the rest is SBUF."""
                oslc_e = ot_f[pr][0:64, qs]
                oslc_o = ot_f[pr][64:128, qs]
                bcs = pN.tile([128, 512], dt.float32, tag="bcs")
                nc.vector.tensor_copy(den0[:], ot_o[DH:DH + 1, :])
                nc.vector.tensor_copy(oslc_o, ot_o[0:DH, :])
                return [
                    lambda: nc.vector.reciprocal_approx_fast(recA[0:1, :], den0[:]),
                    lambda: nc.vector.tensor_copy(den0[:], ot_e[DH:DH + 1, :]),
                    lambda: nc.vector.tensor_copy(oslc_e, ot_e[0:DH, :]),
                    lambda: nc.gpsimd.tensor_copy(recA[64:65, :], recA[0:1, :]),
                    lambda: nc.gpsimd.tensor_copy(recA[96:97, :], recA[0:1, :]),
                    lambda: nc.vector.stream_shuffle(bcs[64:128, :],
                                                     recA[64:128, :], [0] * 32),
                    lambda: nc.vector.tensor_mul(oslc_o, oslc_o, bcs[64:128, :]),
                    lambda: nc.vector.reciprocal_approx_fast(recA[0:1, :], den0[:]),
                    lambda: nc.gpsimd.tensor_copy(recA[32:33, :], recA[0:1, :]),
                    lambda: nc.vector.stream_shuffle(bcs[0:64, :],
                                                     recA[0:64, :], [0] * 32),
                    lambda: nc.vector.tensor_mul(oslc_e, oslc_e, bcs[0:64, :]),
                ]

            dribble = []          # previous pair's normalize, 2 ops/iter
            for pi, (jq, pr) in enumerate(pairs):
                qs = slice(jq * 512, (jq + 1) * 512)
                if pi == 0:
                    inject_at = p0
                    v_unit(0)()  # fully before the kc loop: PV(0) needs it
                else:
                    pend = list(fillers.get(pi, ()))
                    inject_at = {}
                    if pend:
                        # pair 7's O-proj fillers read ot_f written by pair
                        # 6's normalize, which dribbles into iters 0..18
                        # here - delay past it.
                        start = 20 if pi == 7 else 0
                        step = (KC - start) / len(pend)
                        for u, unit in enumerate(pend):
                            inject_at.setdefault(
                                start + int(u * step), []).append(unit)

                ot_e = psO.tile([DH + 1, 512], dt.float32, tag="ote", bufs=2)
                ot_o = psO.tile([DH + 1, 512], dt.float32, tag="oto", bufs=1)
                for kc in range(KC):
                    ks = slice(kc * 128, (kc + 1) * 128)
                    sp = psR.tile([128, 1024], dt.float32, tag="sp")
                    nc.tensor.matmul(sp[:, 0:512], kt_f[pr][0:64, ks],
                                     qt_f[pr][0:64, qs], start=True, stop=True)
                    nc.tensor.matmul(sp[:, 512:1024], kt_f[pr][64:128, ks],
                                     qt_f[pr][64:128, qs], start=True, stop=True)
                    finishers = [unit() for unit in inject_at.get(kc, ())]
                    p_sb = pB.tile([128, 1024], dt.bfloat16, tag="p")
                    nc.scalar.activation(p_sb[:], sp[:], AF.Exp, scale=0.125)
                    pv = p_sb[:].rearrange("p (a b) -> p a b", a=2)
                    mk = msk_sb[:, kc, qs][:, None, :].to_broadcast((128, 2, 512))
                    nc.vector.tensor_mul(pv, pv, mk)
                    nc.tensor.matmul(ot_e[:], v_c[kc][:, 2 * pr, :],
                                     p_sb[:, 0:512],
                                     start=(kc == 0), stop=(kc == KC - 1))
                    nc.tensor.matmul(ot_o[:], v_c[kc][:, 2 * pr + 1, :],
                                     p_sb[:, 512:1024],
                                     start=(kc == 0), stop=(kc == KC - 1))
                    for f in finishers:
                        f()
                    ndrib = 2 if kc == 0 else (1 if kc % 2 == 0 else 0)
                    while dribble and ndrib > 0:
                        dribble.pop(0)()
                        ndrib -= 1

                for op in dribble:    # leftovers (shouldn't happen)
                    op()
                dribble = normalize_ops(pr, qs, ot_e, ot_o)

            # tail: O-proj of the last query block.  dc 0-2 rows of ot_f
            # were normalized pairs ago - accumulate them on the free "sp"
            # ring NOW so the PE works while pair 7's normalize (the dribble
            # list) drains on DVE; only the dc=3 matmul + copy + DMA remain
            # serialized behind the normalize.
            tail_ps = [psR.tile([128, 1024], dt.float32, tag="sp",
                                name=f"tailps{i}")
                       for i in range(2)]
            for j, tt in enumerate(range(4, 8)):
                tps = tail_ps[j // 2]
                half = slice((j % 2) * 512, (j % 2) * 512 + 512)
                for dc in range(3):
                    nc.tensor.matmul(
                        tps[:, half],
                        ot_f[dc][:, tt * 128:(tt + 1) * 128],
                        wo_sb[:, dc, :], start=(dc == 0), stop=False)
            for op in dribble:        # last pair's normalize (DVE)
                op()
            bcs_stage = pN.tile([128, 512], dt.float32, tag="bcs")
            fins = []
            for j, tt in enumerate(range(4, 8)):
                tps = tail_ps[j // 2]
                half = slice((j % 2) * 512, (j % 2) * 512 + 512)
                nc.tensor.matmul(
                    tps[:, half],
                    ot_f[3][:, tt * 128:(tt + 1) * 128],
                    wo_sb[:, 3, :], start=False,
                    stop=(with_bias is False))
                if with_bias:
                    nc.tensor.matmul(tps[:, half], ones1_sb[:],
                                     bvo_sb[:, 1, :], start=False, stop=True)

                def fin(tps=tps, half=half, tt=tt, j=j):
                    os = bcs_stage if j % 2 else                         pC.tile([128, 512], dt.float32, tag="os")
                    nc.vector.tensor_copy(os[:, 0:512], tps[:, half])
                    nc.sync.dma_start(out[tt * 128:(tt + 1) * 128, :],
                                      os[:, 0:512])
                fins.append(fin)
                if j >= 1:
                    fins.pop(0)()
            for f in fins:
                f()

    nc.compile()
    return nc


def _get_nc(with_bias: bool):
    if with_bias not in _BUILT:
        _BUILT[with_bias] = _build(with_bias)
    return _BUILT[with_bias]


def _prep_inputs(x, Wq, bq, Wk, bk, Wv, bv, Wo, bo, mask, with_bias):
    bf16 = ml_dtypes.bfloat16

    shared = {}
    for name, W in (("wq", Wq), ("wk", Wk), ("wv", Wv), ("wo", Wo)):
        shared[name] = np.ascontiguousarray(
            np.asarray(W, np.float32).astype(bf16)
            .reshape(DC, 128, D).transpose(1, 0, 2))
    if with_bias:
        shared["bqk"] = np.ascontiguousarray(np.stack(
            [np.asarray(bq, np.float32).reshape(DC, 128).T,
             np.asarray(bk, np.float32).reshape(DC, 128).T], axis=-1))
        shared["bvo"] = np.ascontiguousarray(np.stack(
            [np.asarray(bv, np.float32), np.asarray(bo, np.float32)]
        ).astype(bf16).reshape(1, 2, D))
        shared["ones1"] = np.ones((1, 128), np.float32).astype(bf16)

    maskT = np.asarray(mask).reshape(T, T).T          # (k, q)
    m01T = maskT.astype(np.float32)

    in_maps = []
    for c in range(N_CORES):
        b, q0 = c // 4, (c % 4) * QPC
        # Roll the token axis so this core's query block sits at columns
        # 0..QPC; keys are consistently permuted (softmax + PV are
        # permutation-invariant over keys); the mask key axis rolls too.
        xTb = np.asarray(x[b], np.float32).T.astype(bf16)     # (D, T)
        xTr = np.roll(xTb, -q0, axis=1)
        m = dict(shared)
        m["xT"] = np.ascontiguousarray(
            xTr.reshape(DC, 128, T).transpose(1, 0, 2))
        mr = np.roll(m01T[:, q0:q0 + QPC], -q0, axis=0)       # (k rolled, q)
        m["msk"] = np.ascontiguousarray(
            mr.reshape(KC, 128, QPC).transpose(1, 0, 2)).astype(bf16)
        in_maps.append(m)
    return in_maps


def _run(inputs, trace=False):
    from concourse.bass_utils import run_bass_kernel_spmd

    with_bias = any(
        float(np.abs(np.asarray(inputs[k], np.float32)).max()) != 0.0
        for k in ("bq", "bk", "bv", "bo"))
    nc = _get_nc(with_bias)
    in_maps = _prep_inputs(
        inputs["x"], inputs["Wq"], inputs["bq"], inputs["Wk"], inputs["bk"],
        inputs["Wv"], inputs["bv"], inputs["Wo"], inputs["bo"],
        inputs["mask"], with_bias)
    res = run_bass_kernel_spmd(nc, in_maps, list(range(N_CORES)), trace=trace)
    O = np.empty((B, T, D), np.float32)
    for c in range(N_CORES):
        b, q0 = c // 4, (c % 4) * QPC
        O[b, q0:q0 + QPC, :] = res.results[c]["out"]
    return O, res


def kernel(**inputs) -> np.ndarray:
    out, _ = _run(inputs, trace=False)
    return out
